# revision 1
# baseline (speedup 1.0000x reference)
"""DiffGuidedFilter (r=1, eps=1e-8) Trainium2 Bass kernel.

Input: guidance, src [8, 3, 1024, 1024] f32. Output: same shape.
Sharding: pure data parallel, one batch element per NeuronCore (8 cores).

Per-core layout: 3 channels x 1024x1024 images, processed as 9 overlapping
128-row tiles per channel (rows = SBUF partitions, columns = free dim).
The separable 3x3 box filters are split: horizontal 3-sums run as shifted
adds on DVE/GPSIMD over zero-padded tiles; vertical 3-sums run on the
TensorEngine as banded 0/1-stationary matmuls (the only cheap way to mix
partitions; fp32 matmul is 4 cycles/row and PE is the critical path, so
every field takes the cheapest 2-pass vertical form).
Count normalization (1/(vc*3)) is applied as per-partition scale vectors
via ACT activation scale and DVE scalar_tensor_tensor, keeping rounding
close to the reference's sum-then-divide ordering. All ACT functions come
from one activation-table set (see _Bacc below) so the table loads once
instead of thrashing every tile. The reciprocal runs on ACT as exp(-ln(var)) with eps
folded into Ln's per-partition bias. The hc=2 column-edge correction (output columns
0,1,1022,1023) is applied on the host in float64.

Engine budget per 128x1024 tile: PE 12 matmul passes, DVE ~14 ops,
ACT ~6 ops, GPSIMD ~6 ops, DMA ~1.5 MiB.
"""
import numpy as np

B, C, H, W = 8, 3, 1024, 1024
P = 128
EPS = 1e-8

_CACHE = {}


def _t1_matrix(variant):
    # stage-1 raw box sum: T[m, k] = 1 for k in {m-1, m, m+1} within range
    T = np.zeros((P, P), np.float32)
    for m in range(P):
        for k in (m - 1, m, m + 1):
            if 0 <= k < P:
                T[m, k] = 1.0
    return T


def _t2_matrix(variant):
    # stage-2 raw box sum over a,b: valid k excludes partitions whose
    # stage-1 value is garbage; rows outside the output range are zeroed.
    if variant == "top":
        k_lo, k_hi, m_lo, m_hi = 0, 127, 0, 126
    elif variant == "mid":
        k_lo, k_hi, m_lo, m_hi = 1, 127, 2, 126
    else:  # bot
        k_lo, k_hi, m_lo, m_hi = 1, 128, 2, 128
    T = np.zeros((P, P), np.float32)
    for m in range(m_lo, m_hi):
        for k in (m - 1, m, m + 1):
            if k_lo <= k < k_hi:
                T[m, k] = 1.0
    return T


def _m_vector(variant):
    # per-partition normalization 1/(3*vc): vc=2 at true image edge rows
    v = np.full(P, 1.0 / 9.0, np.float32)
    if variant == "top":
        v[0] = np.float32(1.0 / 6.0)
    if variant == "bot":
        v[P - 1] = np.float32(1.0 / 6.0)
    return v


def _tile_plan():
    """Per channel: list of (r0, variant, o_lo, o_hi, orow0)."""
    plan = [(0, "top", 0, 126, 0)]
    bounds = np.linspace(126, 898, 8).round().astype(int)
    for i in range(7):
        b0, b1 = int(bounds[i]), int(bounds[i + 1])
        plan.append((b0 - 2, "mid", 2, 2 + (b1 - b0), b0))
    plan.append((H - P, "bot", 2, 128, 898))
    return plan


def _build_program(n_ch=C, n_tiles=None):
    import concourse.bacc as bacc
    import concourse.tile as tile
    from concourse import mybir

    F32 = mybir.dt.float32
    AF = mybir.ActivationFunctionType
    ALU = mybir.AluOpType

    class _Bacc(bacc.Bacc):
        # All four activation funcs used here (Square, Copy, Ln, Exp) live in
        # the natural_log_exp_and_others set; put it first so the table-load
        # inserter picks it for every op instead of thrashing between sets.
        def insert_act_table_loads(self):
            import concourse.hw_specs as hw_specs
            from concourse.bass import _bass_rust
            has_activation = any(
                isinstance(i, mybir.InstActivation)
                for b in self.main_func.blocks
                for i in b.instructions
            )
            if not has_activation:
                return
            tables = list(hw_specs.get_activation_tables(self.m.arch).items())
            # act_func_set_id is positional: keep list order, but empty out
            # every other set so the chooser can only pick this one.
            tables = [(n, fs if n == "natural_log_exp_and_others" else set())
                      for n, fs in tables]
            _bass_rust.insert_act_table_loads(self, tables)

    nc = _Bacc()
    g_in = nc.dram_tensor("g", [C, H, W], F32, kind="ExternalInput")
    s_in = nc.dram_tensor("s", [C, H, W], F32, kind="ExternalInput")
    tm_in = nc.dram_tensor("tm", [6, P, P], F32, kind="ExternalInput")
    mv_in = nc.dram_tensor("mv", [6, P], F32, kind="ExternalInput")
    o_out = nc.dram_tensor("o", [C, H, W], F32, kind="ExternalOutput")

    WP = W + 2  # padded width
    VAR_IDX = {"top": 0, "mid": 1, "bot": 2}
    plan = _tile_plan()

    with tile.TileContext(nc) as tc:
        with tc.tile_pool(name="const", bufs=1) as constp, \
             tc.tile_pool(name="big", bufs=2) as bigp, \
             tc.tile_pool(name="small", bufs=2) as smp, \
             tc.tile_pool(name="psum", bufs=4, space="PSUM") as psp:

            tmt = constp.tile([P, 6 * P], F32, tag="tmt")
            for i in range(6):
                nc.sync.dma_start(tmt[:, i * P:(i + 1) * P], tm_in[i])
            mvt = constp.tile([P, 6], F32, tag="mvt")
            for i in range(6):
                nc.sync.dma_start(mvt[:, i:i + 1], mv_in[i])
            epst = constp.tile([P, 1], F32, tag="epst")
            nc.vector.memset(epst[:], EPS)

            # one-time pad-column zeroing per pool slot (cols 0 and W+1 are
            # never written inside the loop)
            for tag in ["gpad", "spad", "ggpad", "gspad", "apad", "bpad"]:
                nb = 3 if tag in ("gpad", "spad", "ggpad", "gspad") else 2
                for buf in range(nb):
                    t = bigp.tile([P, WP], F32, tag=tag, name=f"init_{tag}",
                                  bufs=nb)
                    nc.vector.memset(t[:, 0:1], 0.0)
                    nc.vector.memset(t[:, W + 1:W + 2], 0.0)

            for ch in range(n_ch):
                for (r0, var, o_lo, o_hi, orow0) in \
                        (plan if n_tiles is None else plan[:n_tiles]):
                    i1 = VAR_IDX[var]
                    i2 = 3 + i1
                    tm1 = tmt[:, i1 * P:(i1 + 1) * P]
                    tm2 = tmt[:, i2 * P:(i2 + 1) * P]
                    m1 = mvt[:, i1:i1 + 1]
                    m2 = mvt[:, i2:i2 + 1]

                    gpad = bigp.tile([P, WP], F32, tag="gpad", bufs=3)
                    spad = bigp.tile([P, WP], F32, tag="spad", bufs=3)
                    nc.sync.dma_start(gpad[:, 1:W + 1], g_in[ch, r0:r0 + P, :])
                    nc.sync.dma_start(spad[:, 1:W + 1], s_in[ch, r0:r0 + P, :])

                    ggpad = bigp.tile([P, WP], F32, tag="ggpad", bufs=3)
                    gspad = bigp.tile([P, WP], F32, tag="gspad", bufs=3)
                    nc.scalar.activation(ggpad[:, 1:W + 1], gpad[:, 1:W + 1],
                                         AF.Square)
                    nc.gpsimd.tensor_mul(gspad[:, 1:W + 1], gpad[:, 1:W + 1],
                                         spad[:, 1:W + 1])

                    # horizontal 3-sums on DVE/GPS ([128, W] tiles).
                    # g: both adds on DVE (second in-place)
                    hg = smp.tile([P, W], F32, tag="hg")
                    nc.vector.tensor_add(hg[:], gpad[:, 0:W], gpad[:, 1:W + 1])
                    nc.vector.tensor_add(hg[:], hg[:], gpad[:, 2:W + 2])
                    # s: both adds on DVE (PE passes are the critical path)
                    hs = smp.tile([P, W], F32, tag="hs")
                    nc.vector.tensor_add(hs[:], spad[:, 0:W], spad[:, 1:W + 1])
                    nc.vector.tensor_add(hs[:], hs[:], spad[:, 2:W + 2])
                    # gg: first add on GPS, second on DVE
                    hgg1 = smp.tile([P, W], F32, tag="h1t", name="hgg1", bufs=3)
                    nc.gpsimd.tensor_add(hgg1[:], ggpad[:, 0:W],
                                         ggpad[:, 1:W + 1])
                    hgg = smp.tile([P, W], F32, tag="hgg")
                    nc.vector.tensor_add(hgg[:], hgg1[:], ggpad[:, 2:W + 2])
                    # gs: first add on GPS, second on DVE
                    hgs1 = smp.tile([P, W], F32, tag="h1t", name="hgs1", bufs=3)
                    nc.gpsimd.tensor_add(hgs1[:], gspad[:, 0:W],
                                         gspad[:, 1:W + 1])
                    hgs = smp.tile([P, W], F32, tag="hgs")
                    nc.vector.tensor_add(hgs[:], hgs1[:], gspad[:, 2:W + 2])

                    # stage 1 vertical sums on PE (band stationary)
                    ps1 = [psp.tile([P, W], F32, tag="ps", name=f"ps1_{f_}")
                           for f_ in range(4)]
                    Sg, Ss, Sgg, Sgs = ps1
                    for c in range(2):
                        sl = slice(c * 512, (c + 1) * 512)
                        nc.tensor.matmul(Sg[:, sl], tm1, hg[:, sl],
                                         start=True, stop=True)
                        nc.tensor.matmul(Ss[:, sl], tm1, hs[:, sl],
                                         start=True, stop=True)
                        nc.tensor.matmul(Sgg[:, sl], tm1, hgg[:, sl],
                                         start=True, stop=True)
                        nc.tensor.matmul(Sgs[:, sl], tm1, hgs[:, sl],
                                         start=True, stop=True)

                    # combine: means via per-partition scale m1
                    mx2 = smp.tile([P, W], F32, tag="dl", name="mx2", bufs=4)
                    mxe = smp.tile([P, W], F32, tag="mxe")
                    nc.scalar.activation(mx2[:], Sg[:], AF.Square, scale=m1)
                    nc.scalar.activation(mxe[:], Sg[:], AF.Copy, scale=m1)

                    # den = var = mean_xx - mean_x^2; eps folds into Ln's bias
                    den = smp.tile([P, W], F32, tag="dl", name="den", bufs=4)
                    nc.vector.scalar_tensor_tensor(
                        den[:], Sgg[:], m1, mx2[:],
                        op0=ALU.mult, op1=ALU.subtract)
                    lgd = smp.tile([P, W], F32, tag="dl", name="lgd", bufs=4)
                    nc.scalar.activation(lgd[:], den[:], AF.Ln, bias=epst[:, 0:1])
                    rec = smp.tile([P, W], F32, tag="rec")
                    nc.scalar.activation(rec[:], lgd[:], AF.Exp, scale=-1.0)

                    # t4 = mean_y * mean_x = (Ss*m1) * mxe
                    t4 = smp.tile([P, W], F32, tag="t4")
                    nc.vector.scalar_tensor_tensor(
                        t4[:], Ss[:], m1, mxe[:], op0=ALU.mult, op1=ALU.mult)
                    # cov = (Sgs*m1) - t4
                    cov = smp.tile([P, W], F32, tag="cov")
                    nc.vector.scalar_tensor_tensor(
                        cov[:], Sgs[:], m1, t4[:],
                        op0=ALU.mult, op1=ALU.subtract)

                    apad = bigp.tile([P, WP], F32, tag="apad")
                    bpad = bigp.tile([P, WP], F32, tag="bpad")
                    nc.gpsimd.tensor_mul(apad[:, 1:W + 1], cov[:], rec[:])
                    t6 = smp.tile([P, W], F32, tag="t6")
                    nc.gpsimd.tensor_mul(t6[:], apad[:, 1:W + 1], mxe[:])
                    # b = mean_y - t6 = (Ss*m1) - t6
                    nc.vector.scalar_tensor_tensor(
                        bpad[:, 1:W + 1], Ss[:], m1, t6[:],
                        op0=ALU.mult, op1=ALU.subtract)

                    # horizontal 3-sums of a and b
                    ha = smp.tile([P, W], F32, tag="ha")
                    nc.vector.tensor_add(ha[:], apad[:, 0:W], apad[:, 1:W + 1])
                    nc.vector.tensor_add(ha[:], ha[:], apad[:, 2:W + 2])
                    hb1 = smp.tile([P, W], F32, tag="h1t", name="hb1", bufs=3)
                    nc.gpsimd.tensor_add(hb1[:], bpad[:, 0:W],
                                         bpad[:, 1:W + 1])
                    hb = smp.tile([P, W], F32, tag="hb")
                    nc.vector.tensor_add(hb[:], hb1[:], bpad[:, 2:W + 2])

                    # stage 2 vertical sums on PE
                    ps2 = [psp.tile([P, W], F32, tag="ps", name=f"ps2_{f_}")
                           for f_ in range(2)]
                    MA, MB = ps2
                    for c in range(2):
                        sl = slice(c * 512, (c + 1) * 512)
                        nc.tensor.matmul(MA[:, sl], tm2, ha[:, sl],
                                         start=True, stop=True)
                        nc.tensor.matmul(MB[:, sl], tm2, hb[:, sl],
                                         start=True, stop=True)

                    # out = (MA*m2)*g + (MB*m2)
                    t5 = smp.tile([P, W], F32, tag="t5")
                    nc.vector.scalar_tensor_tensor(
                        t5[:], MA[:], m2, gpad[:, 1:W + 1],
                        op0=ALU.mult, op1=ALU.mult)
                    outt = smp.tile([P, W], F32, tag="outt")
                    nc.vector.scalar_tensor_tensor(
                        outt[:], MB[:], m2, t5[:],
                        op0=ALU.mult, op1=ALU.add)

                    nrows = o_hi - o_lo
                    nc.sync.dma_start(o_out[ch, orow0:orow0 + nrows, :],
                                      outt[o_lo:o_hi, :])

    nc.finalize()
    return nc


def _make_consts():
    tm = np.stack([
        _t1_matrix("top").T, _t1_matrix("mid").T, _t1_matrix("bot").T,
        _t2_matrix("top").T, _t2_matrix("mid").T, _t2_matrix("bot").T,
    ]).copy()
    mv = np.stack([
        _m_vector("top"), _m_vector("mid"), _m_vector("bot"),
        _m_vector("top"), _m_vector("mid"), _m_vector("bot"),
    ]).copy()
    return tm, mv


def _host_edge_fix(g, s, out):
    """Recompute output cols {0,1,1022,1023} (hc=2 edge normalization) on
    the host in float64. g, s: [B, C, H, W] float32; out modified in place.
    """
    def fix(gs_cols, ss_cols, left):
        g64 = gs_cols.astype(np.float64)
        s64 = ss_cols.astype(np.float64)

        def wsum(x):
            xp = np.pad(x, [(0, 0)] * (x.ndim - 2) + [(1, 1), (1, 1)])
            v = xp[..., :-2, :] + xp[..., 1:-1, :] + xp[..., 2:, :]
            return v[..., :-2] + v[..., 1:-1] + v[..., 2:]

        cnt = wsum(np.ones_like(g64))
        mean_x = wsum(g64) / cnt
        mean_y = wsum(s64) / cnt
        mean_xx = wsum(g64 * g64) / cnt
        mean_xy = wsum(g64 * s64) / cnt
        var = mean_xx - mean_x * mean_x
        cov = mean_xy - mean_x * mean_y
        a = cov / (var + EPS)
        b = mean_y - a * mean_x
        mean_a = wsum(a) / cnt
        mean_b = wsum(b) / cnt
        res = mean_a * g64 + mean_b
        return res[..., 0:2] if left else res[..., -2:]

    out[..., 0:2] = fix(g[..., 0:5], s[..., 0:5], True).astype(np.float32)
    out[..., W - 2:W] = fix(g[..., W - 5:W], s[..., W - 5:W],
                            False).astype(np.float32)


def kernel(guidance, src):
    from concourse.bass_utils import run_bass_kernel_spmd

    g = np.ascontiguousarray(np.asarray(guidance, dtype=np.float32))
    s = np.ascontiguousarray(np.asarray(src, dtype=np.float32))

    if "nc" not in _CACHE:
        _CACHE["nc"] = _build_program()
    nc = _CACHE["nc"]

    tm, mv = _make_consts()
    in_maps = [{"g": g[b], "s": s[b], "tm": tm, "mv": mv} for b in range(B)]
    res = run_bass_kernel_spmd(nc, in_maps, core_ids=list(range(B)))
    out = np.stack([res.results[b]["o"] for b in range(B)])

    _host_edge_fix(g, s, out)
    return out

